# revision 8
# baseline (speedup 1.0000x reference)
"""Trainium2 Bass kernel for BinaryDecoderV2.

Computes loss = mean(((latent @ int_weights) - int_sum)^2) / 255^2 where
int_weights packs sign bits of `weight` into two's-complement int8 and
int_sum packs `true_sum` bit-planes the same way.

Sharding: 2D grid over 8 NeuronCores — 4 batch shards x 2 out_features
shards. Core c owns batch rows [br*512, (br+1)*512) and output columns
[oc*512, (oc+1)*512) with br = c // 2, oc = c % 2. No collectives —
each core emits [128, 4] partial sums of squared diffs; the host
reduces them to the scalar loss.

Host prep (pure repack/quantize):
  - int_w = packbits(weight > 0) viewed as int8 == the reference's
    two's-complement einsum pack, exactly. Shipped NEGATED as fp8e4m3.
  - int_sum = true_sum bit-plane pack (f32 einsum), shipped as fp8e4m3
    together with a 128x128 identity in one `aux` tensor.
  - latent shipped as fp8e4m3.

Per core (vs the 101us baseline: no on-device bit packing at all, fp8
DoubleRow matmuls at 2x bf16 rate, 8.3MB DMA instead of 27.5MB):
  - psum[ob] (4 banks of [128, 512] f32) accumulates latent @ (-int_w)
    over 32 DoubleRow fp8 matmuls per bank (each contracts TWO k-tiles
    at double pump), then +int_sum is added at the END of each chain via
    an identity matmul (runs on a warm PE, keeps the tiny aux DMA off
    the critical path). psum = int_sum - pred = -diff.
  - dummy warm-up matmuls on a memset tile ramp the PE p-state while
    the first latent/weight chunks are still in flight.
  - input DMAs are issued from four queues in parallel (sync/vector/
    scalar/gpsimd) with small leading chunks so the matmul stream
    starts ~2us after the framework preamble.
  - loss partial via ACT Square+accum_out straight from PSUM -> [128,4],
    interleaved ob-major in the last chunk so only the final bank's
    ACT is a serial tail.
"""

import numpy as np
import ml_dtypes

IN_FEATURES = 8192
OUT_FEATURES = 1024
N_BITS = 8
BATCH = 2048
N_CORES = 8
BR = 4                      # batch shards
OC = 2                      # out_features shards
NB = BATCH // BR            # 512 batch rows per core
OO = OUT_FEATURES // OC     # 512 outputs per core
KP = 128                    # k per tile (partition dim)
KT = IN_FEATURES // KP      # 64 k-tiles
OBLK = OO // 128            # 4 out blocks (psum banks) per core
CHUNK_KT = [2, 6, 24, 32]   # k-tiles per DMA chunk (even: DR pairs)
TAIL_KT = 6                 # k-tiles emitted ob-major at the very end
N_WARM = 14                 # PE p-state warm-up matmuls (~3us continuous)
SCALE = 2.0 ** N_BITS - 1.0
POWERS = [1.0, 2.0, 4.0, 8.0, 16.0, 32.0, 64.0, -128.0]

_CACHE: dict = {}


def _build():
    import concourse.bacc as bacc
    import concourse.mybir as mybir
    from concourse import tile

    f8e4 = mybir.dt.float8e4
    f32 = mybir.dt.float32
    Act = mybir.ActivationFunctionType
    DR = mybir.MatmulPerfMode.DoubleRow

    nc = bacc.Bacc("TRN2", target_bir_lowering=False, debug=False,
                   num_devices=N_CORES)

    latq = nc.dram_tensor("latq", [128, KT, NB], f8e4, kind="ExternalInput")
    wq = nc.dram_tensor("wq", [128, KT, OO], f8e4, kind="ExternalInput")
    # aux = [identity(128) | int_sum planes ob=0..3]
    aux = nc.dram_tensor("aux", [128, 128 + OBLK * NB], f8e4,
                         kind="ExternalInput")
    partials = nc.dram_tensor("partials", [128, OBLK], f32,
                              kind="ExternalOutput")

    with tile.TileContext(nc) as tc:
        with (
            tc.tile_pool(name="wp", bufs=1) as w_pool,
            tc.tile_pool(name="lp", bufs=1) as l_pool,
            tc.tile_pool(name="aux", bufs=1) as aux_pool,
            tc.tile_pool(name="warm", bufs=1) as warm_pool,
            tc.tile_pool(name="sq", bufs=2) as sq_pool,
            tc.tile_pool(name="loss", bufs=1) as loss_pool,
            tc.tile_pool(name="ps", bufs=1, space="PSUM") as psum_pool,
        ):
            # ---- PE p-state warm-up on a memset tile (no data deps) ----
            warm = warm_pool.tile([128, 2, 256], f8e4)
            nc.gpsimd.memset(warm[:], 0)

            # ---- input DMAs; single issue queue => completion follows
            # issue order (hw engines round-robin packets of in-flight
            # transfers, so ordering is what guarantees prefix arrival) --
            wts, lts = [], []
            s = 0
            for ci, n in enumerate(CHUNK_KT):
                wt = w_pool.tile([128, n, OO], f8e4, name=f"w{ci}",
                                 tag=f"w{ci}")
                nc.sync.dma_start(wt[:], wq[:, s:s + n, :])
                lt = l_pool.tile([128, n, NB], f8e4, name=f"l{ci}",
                                 tag=f"l{ci}")
                nc.sync.dma_start(lt[:], latq[:, s:s + n, :])
                wts.append((s, n, wt))
                lts.append(lt)
                s += n
                if ci == 1:  # aux is tiny and only needed at the tail
                    ax = aux_pool.tile([128, 128 + OBLK * NB], f8e4)
                    nc.sync.dma_start(ax[:], aux[:])
            wps = psum_pool.tile([128, 256], f32, name="wps", tag="wps")
            for _ in range(N_WARM):
                nc.tensor.matmul(wps[:], warm[:, :, 0:128], warm[:],
                                 start=True, stop=True, perf_mode=DR)

            # ---- psum[ob] = -pred: fp8 DoubleRow (2 k-tiles each) ----
            psums = [psum_pool.tile([128, NB], f32, name=f"ps{i}",
                                    tag=f"ps{i}") for i in range(OBLK)]
            out_t = loss_pool.tile([128, OBLK], f32)
            last = len(CHUNK_KT) - 1
            for ci, (cs, cn, wt) in enumerate(wts):
                lt = lts[ci]
                # kp-major: all banks advance together (chunk can be
                # consumed as it streams in); the final TAIL_KT k-tiles
                # switch to ob-major so banks finish one by one and the
                # diag preload + ACT overlap the remaining matmuls
                head = cn if ci < last else cn - TAIL_KT
                for j in range(0, head, 2):
                    for ob in range(OBLK):
                        nc.tensor.matmul(
                            psums[ob][:],
                            wt[:, j:j + 2, ob * 128:(ob + 1) * 128],
                            lt[:, j:j + 2, :],
                            start=(cs + j == 0), stop=False,
                            perf_mode=DR)
                if ci == last:
                    for ob in range(OBLK):
                        for j in range(head, cn, 2):
                            nc.tensor.matmul(
                                psums[ob][:],
                                wt[:, j:j + 2, ob * 128:(ob + 1) * 128],
                                lt[:, j:j + 2, :],
                                start=False, stop=False, perf_mode=DR)
                        # psum[ob] += int_sum (identity matmul, warm PE)
                        nc.tensor.matmul(
                            psums[ob][:], ax[:, 0:128],
                            ax[:, 128 + ob * NB:128 + (ob + 1) * NB],
                            start=False, stop=True)
                        # partial[o, ob] = sum_n diff^2 (ACT from PSUM)
                        d2 = sq_pool.tile([128, NB], f32, name=f"d2_{ob}",
                                          tag="d2")
                        nc.scalar.activation(d2[:], psums[ob][:], Act.Square,
                                             accum_out=out_t[:, ob:ob + 1])

            nc.sync.dma_start(partials[:], out_t[:])

    nc.compile()
    return nc


def _get_nc():
    if "nc" not in _CACHE:
        _CACHE["nc"] = _build()
    return _CACHE["nc"]


def make_in_maps(latent: np.ndarray, true_sum: np.ndarray,
                 weight: np.ndarray) -> list:
    f8 = ml_dtypes.float8_e4m3fn

    # latq[p, kt, n] = latent[n, kt*128 + p], sliced per batch shard
    lat8 = latent.astype(f8)
    latq = lat8.T.reshape(KT, KP, BATCH).transpose(1, 0, 2)  # [128, KT, B]
    latqs = [np.ascontiguousarray(latq[:, :, br * NB:(br + 1) * NB])
             for br in range(BR)]

    # int_w[k, o] = two's-complement pack of sign bits; ship -int_w fp8
    bits = (weight > 0).reshape(IN_FEATURES, OUT_FEATURES, N_BITS)
    intw = np.packbits(bits, axis=-1, bitorder="little")[..., 0]
    nw = -intw.view(np.int8).astype(np.float32)             # [K, O]
    nwq = nw.reshape(KT, KP, OUT_FEATURES).transpose(1, 0, 2)  # [128, KT, O]
    nwqs = [np.ascontiguousarray(nwq[:, :, oc * OO:(oc + 1) * OO]).astype(f8)
            for oc in range(OC)]

    # int_sum[n, o]; per core aux[p, 128 + ob*NB + n], o = ob*128 + p
    powers = np.array(POWERS, dtype=np.float32)
    ts = true_sum.reshape(BATCH, OUT_FEATURES, N_BITS) @ powers  # [B, O]
    tsT = ts.T                                               # [O, B]

    eye = np.eye(128, dtype=np.float32)

    in_maps = []
    for c in range(N_CORES):
        br, oc = c // OC, c % OC
        t = tsT[oc * OO:(oc + 1) * OO, br * NB:(br + 1) * NB]
        tq = t.reshape(OBLK, 128, NB).transpose(1, 0, 2).reshape(128, -1)
        ax = np.concatenate([eye, tq], axis=1).astype(f8)
        in_maps.append({"latq": latqs[br], "wq": nwqs[oc],
                        "aux": np.ascontiguousarray(ax)})
    return in_maps


def kernel(latent: np.ndarray, true_sum: np.ndarray,
           weight: np.ndarray) -> np.ndarray:
    from concourse.bass_utils import run_bass_kernel_spmd

    nc = _get_nc()
    in_maps = make_in_maps(latent, true_sum, weight)
    res = run_bass_kernel_spmd(nc, in_maps, list(range(N_CORES)))

    total = 0.0
    for c in range(N_CORES):
        total += float(res.results[c]["partials"].astype(np.float64).sum())
    loss = total / (BATCH * OUT_FEATURES) / (SCALE * SCALE)
    return np.array(loss, dtype=np.float32)
